# revision 10
# baseline (speedup 1.0000x reference)
"""Trainium2 Bass kernel for GCNNetwork (GENConv message passing, L=6).

Strategy (graph-data parallel over 8 NeuronCores):
 - Nodes sharded contiguously: core c owns rows [7552c, 7552c+7552) (N padded
   60000 -> 60416). Edges assigned to the core owning their dst, sorted by dst.
 - Per layer: LayerNorm on the local shard -> AllGather y across cores (y_full)
   -> per 128-edge tile: indirect-DMA gather y[src], compute msg/w=exp(msg)/
   msg*w, segment-reduce to the 128-node block via an indicator matmul
   accumulated in PSUM ([denom | numer]) -> softmax-aggregate, residual,
   conv matmul, relu -> pooling via batch-indicator matmul accumulated in PSUM.
 - Pool windows (128 graphs/core) are indirect-scattered into a global
   [3072+, 256] z buffer, AllReduced, and the readout MLP is computed
   replicated on every core.

All weights are replicated. Biases/LN affine params are applied only when
nonzero/non-one (they are zeros/ones for this model's init).
"""
import sys
import numpy as np

for _p in ("/opt/trn_rl_repo", "/root/.axon_site/_ro/trn_rl_repo"):
    if _p not in sys.path:
        sys.path.append(_p)

import concourse.bass as bass
import concourse.bacc as bacc
import concourse.mybir as mybir
import concourse.tile as tile
from concourse.bass_utils import run_bass_kernel_spmd

F32 = mybir.dt.float32
I32 = mybir.dt.int32
ALU = mybir.AluOpType
ACTF = mybir.ActivationFunctionType

N, E, B, D, L = 60000, 120000, 512, 256, 6
NTYPES = 25
LN_EPS = 1e-5
NC = 8
NBLK = 59                 # 128-node blocks per core
NSHARD = NBLK * 128       # 7552
ZROWS = 3200              # z buffer rows (L*512 = 3072 used, row 3072 = dump)
ZDUMP = 3072

# module-level knobs (test.py pokes these; harness uses defaults)
TRACE = False
TRACE_CORES = None
LAST_RESULT = {}

_prog_cache = {}


def _ceil_div(a, b):
    return (a + b - 1) // b


# ----------------------------------------------------------------------------
# host-side preprocessing
# ----------------------------------------------------------------------------

def _prep(inputs):
    x = np.asarray(inputs["x"]).astype(np.int32).reshape(-1)
    ei = np.asarray(inputs["edge_index"]).astype(np.int64)
    ea = np.asarray(inputs["edge_attr"]).astype(np.float32).reshape(-1)
    batch = np.asarray(inputs["batch"]).astype(np.int64).reshape(-1)
    src_all, dst_all = ei[0], ei[1]

    # ---- per-core edge partition (by dst), sorted by dst ----
    per_core = []
    cnts = np.zeros((NC, NBLK), dtype=np.int64)
    for c in range(NC):
        lo, hi = c * NSHARD, min((c + 1) * NSHARD, N)
        sel = (dst_all >= lo) & (dst_all < hi)
        d = dst_all[sel]
        s = src_all[sel]
        a = ea[sel]
        o = np.argsort(d, kind="stable")
        d, s, a = d[o], s[o], a[o]
        blk = (d - lo) >> 7
        cnts[c] = np.bincount(blk, minlength=NBLK)
        per_core.append((d, s, a, blk))

    tiles_b = np.maximum(1, _ceil_div(cnts.max(axis=0), 128)).astype(np.int64)
    tile_start = np.concatenate([[0], np.cumsum(tiles_b)])
    T = int(tile_start[-1])
    T4 = _ceil_div(T, 4)
    TT = T4 * 4
    block_of_tile = np.repeat(np.arange(NBLK), tiles_b)

    in_maps = []
    for c in range(NC):
        lo, hi = c * NSHARD, min((c + 1) * NSHARD, N)
        d, s, a, blk = per_core[c]
        ne = len(d)
        # slot of each edge: tile_start[blk]*128 + rank within block
        bstart = np.concatenate([[0], np.cumsum(cnts[c])])
        rank = np.arange(ne) - np.repeat(bstart[:-1], cnts[c])
        slot = tile_start[blk] * 128 + rank
        esrc = np.zeros(TT * 128, dtype=np.int32)
        dstl = np.full(TT * 128, -1.0, dtype=np.float32)
        eav = np.zeros(TT * 128, dtype=np.float32)
        esrc[slot] = s
        dstl[slot] = (d - lo - (blk << 7)).astype(np.float32)
        eav[slot] = a
        # [TT,128] -> [T4,128,4]
        esrc4 = esrc.reshape(T4, 4, 128).transpose(0, 2, 1).copy()
        dstl4 = dstl.reshape(T4, 4, 128).transpose(0, 2, 1)
        eav4 = eav.reshape(T4, 4, 128).transpose(0, 2, 1)
        emeta = np.concatenate([dstl4, eav4], axis=2).astype(np.float32).copy()

        # ---- pooling / nodes ----
        g0 = int(batch[lo])
        bl = np.full(NSHARD, -1.0, dtype=np.float32)
        bl[: hi - lo] = (batch[lo:hi] - g0).astype(np.float32)
        assert bl.max() < 128, "graph window exceeds 128 per core"
        batchl = bl.reshape(NBLK, 128, 1)
        xid = np.zeros(NSHARD, dtype=np.int32)
        xid[: hi - lo] = x[lo:hi]
        xid = xid.reshape(NBLK, 128, 1)
        zrow = np.zeros((L, 128, 1), dtype=np.int32)
        for i in range(L):
            g = g0 + np.arange(128)
            zrow[i, :, 0] = np.where(g < B, 512 * i + g, ZDUMP)

        in_maps.append(
            dict(esrc=esrc4, emeta=emeta, batchl=batchl, xid=xid, zrow=zrow)
        )

    # ---- shared weights ----
    wl_w = np.asarray(inputs["wl_w"]).astype(np.float32)     # [L,1,D]
    conv_w = np.asarray(inputs["conv_w"]).astype(np.float32)  # [L,D,D]
    node_emb = np.asarray(inputs["node_emb"]).astype(np.float32)
    ln_scale = np.asarray(inputs["ln_scale"]).astype(np.float32)
    ln_bias = np.asarray(inputs["ln_bias"]).astype(np.float32)
    wl_b = np.asarray(inputs["wl_b"]).astype(np.float32)
    conv_b = np.asarray(inputs["conv_b"]).astype(np.float32)
    ro_w = [np.asarray(inputs[f"ro_w{i}"]).astype(np.float32) for i in range(4)]
    ro_b = [np.asarray(inputs[f"ro_b{i}"]).astype(np.float32) for i in range(4)]

    flags = dict(
        ln_affine=not (np.all(ln_scale == 1.0) and np.all(ln_bias == 0.0)),
        wl_b=bool(np.any(wl_b != 0.0)),
        conv_b=bool(np.any(conv_b != 0.0)),
        ro_b=any(np.any(b != 0.0) for b in ro_b),
    )

    shared = dict(
        wlw=np.repeat(wl_w.reshape(L, 1, D), 128, axis=1).copy(),
        convw=conv_w,
        nemb=node_emb,
        colidx=np.tile(np.arange(128, dtype=np.float32), (128, 1)).copy(),
        ident=np.eye(128, dtype=np.float32),
        row0=ro_w[0], row1=ro_w[1], row2=ro_w[2], row3=ro_w[3],
    )
    if flags["ln_affine"]:
        shared["lnsc"] = np.repeat(ln_scale.reshape(L, 1, D), 128, axis=1).copy()
        shared["lnbs"] = np.repeat(ln_bias.reshape(L, 1, D), 128, axis=1).copy()
    if flags["wl_b"]:
        shared["wlb"] = np.repeat(wl_b.reshape(L, 1, D), 128, axis=1).copy()
    if flags["conv_b"]:
        shared["convb"] = np.repeat(conv_b.reshape(L, 1, D), 128, axis=1).copy()
    if flags["ro_b"]:
        for i, b in enumerate(ro_b):
            shared[f"rob{i}"] = np.repeat(b.reshape(1, -1), 128, axis=0).copy()

    for m in in_maps:
        m.update(shared)
    return in_maps, tiles_b, block_of_tile, T, T4, flags


# ----------------------------------------------------------------------------
# device program
# ----------------------------------------------------------------------------

def _build(tiles_b, block_of_tile, T, T4, flags):
    nc = bacc.Bacc("TRN2", target_bir_lowering=False, debug=False, num_devices=NC)

    # register LN epsilon as a const AP (activation float biases need one)
    _eps_t = nc.alloc_sbuf_tensor(f"const-float32-lneps", [128, 1], F32)
    nc.gpsimd.memset(_eps_t.ap(), LN_EPS)
    nc.const_aps.aps[(F32, LN_EPS)] = _eps_t.ap()
    nc.all_engine_barrier()

    # inputs
    esrc = nc.dram_tensor("esrc", [T4, 128, 4], I32, kind="ExternalInput")
    emeta = nc.dram_tensor("emeta", [T4, 128, 8], F32, kind="ExternalInput")
    batchl = nc.dram_tensor("batchl", [NBLK, 128, 1], F32, kind="ExternalInput")
    xid = nc.dram_tensor("xid", [NBLK, 128, 1], I32, kind="ExternalInput")
    zrow = nc.dram_tensor("zrow", [L, 128, 1], I32, kind="ExternalInput")
    wlw = nc.dram_tensor("wlw", [L, 128, D], F32, kind="ExternalInput")
    convw = nc.dram_tensor("convw", [L, D, D], F32, kind="ExternalInput")
    nemb = nc.dram_tensor("nemb", [NTYPES, D], F32, kind="ExternalInput")
    colidx = nc.dram_tensor("colidx", [128, 128], F32, kind="ExternalInput")
    ident = nc.dram_tensor("ident", [128, 128], F32, kind="ExternalInput")
    row0 = nc.dram_tensor("row0", [6 * D, 768], F32, kind="ExternalInput")
    row1 = nc.dram_tensor("row1", [768, 384], F32, kind="ExternalInput")
    row2 = nc.dram_tensor("row2", [384, 192], F32, kind="ExternalInput")
    row3 = nc.dram_tensor("row3", [192, 1], F32, kind="ExternalInput")
    lnsc = lnbs = wlb = convb = None
    if flags["ln_affine"]:
        lnsc = nc.dram_tensor("lnsc", [L, 128, D], F32, kind="ExternalInput")
        lnbs = nc.dram_tensor("lnbs", [L, 128, D], F32, kind="ExternalInput")
    if flags["wl_b"]:
        wlb = nc.dram_tensor("wlb", [L, 128, D], F32, kind="ExternalInput")
    if flags["conv_b"]:
        convb = nc.dram_tensor("convb", [L, 128, D], F32, kind="ExternalInput")
    robs = None
    if flags["ro_b"]:
        robs = [
            nc.dram_tensor(f"rob{i}", [128, n], F32, kind="ExternalInput")
            for i, n in enumerate([768, 384, 192, 1])
        ]

    out = nc.dram_tensor("out", [B, 1], F32, kind="ExternalOutput")

    # LN groups of up to 4 blocks
    groups = []
    b0 = 0
    while b0 < NBLK:
        nb = min(4, NBLK - b0)
        groups.append((b0, nb))
        b0 += nb

    # tile ranges per block
    tstart = np.concatenate([[0], np.cumsum(tiles_b)]).astype(int)

    with tile.TileContext(nc) as tc:
        with (
            tc.tile_pool(name="dram", bufs=1, space="DRAM") as dram,
            tc.tile_pool(name="consts", bufs=1) as cpool,
            tc.tile_pool(name="lweights", bufs=2) as wpool,
        ):
            hA = dram.tile([NSHARD, D], F32, tag="hA")
            hB = dram.tile([NSHARD, D], F32, tag="hB")
            y_c = dram.tile([NSHARD, D], F32, tag="y_c")
            y_fulls = [
                dram.tile([NC * NSHARD, D], F32, tag=f"y_full_{i}",
                          name=f"y_full_{i}", addr_space="Shared")
                for i in range(L)
            ]
            z_all = dram.tile([ZROWS, D], F32, tag="z_all")
            z_red = dram.tile([ZROWS, D], F32, tag="z_red", addr_space="Shared")

            colidx_t = cpool.tile([128, 128], F32, tag="colidx")
            nc.sync.dma_start(out=colidx_t[:], in_=colidx[:])
            ident_t = cpool.tile([128, 128], F32, tag="ident")
            nc.sync.dma_start(out=ident_t[:], in_=ident[:])

            # ---------- zero z_all ----------
            with tc.tile_pool(name="zz", bufs=1) as zz:
                zt = zz.tile([128, D], F32)
                nc.vector.memset(zt[:], 0.0)
                for k in range(ZROWS // 128):
                    nc.sync.dma_start(
                        out=z_all[k * 128:(k + 1) * 128, :], in_=zt[:]
                    )

            # ---------- embedding ----------
            with tc.tile_pool(name="emb", bufs=4) as ep:
                for b in range(NBLK):
                    xt = ep.tile([128, 1], I32, tag="xt")
                    nc.sync.dma_start(out=xt[:], in_=xid[b])
                    et = ep.tile([128, D], F32, tag="et")
                    nc.gpsimd.indirect_dma_start(
                        out=et[:], out_offset=None, in_=nemb[:],
                        in_offset=bass.IndirectOffsetOnAxis(ap=xt[:, :1], axis=0),
                    )
                    nc.sync.dma_start(
                        out=hA[b * 128:(b + 1) * 128, :], in_=et[:]
                    )

            # ---------- layers ----------
            with (
                tc.tile_pool(name="ln", bufs=2) as lp,
                tc.tile_pool(name="edge", bufs=4) as xp,
                tc.tile_pool(name="blk", bufs=3) as bp,
                tc.tile_pool(name="ps_nd", bufs=3, space="PSUM") as ps_nd,
                tc.tile_pool(name="ps_xt", bufs=2, space="PSUM") as ps_xt,
                tc.tile_pool(name="ps_h", bufs=1, space="PSUM") as ps_h,
                tc.tile_pool(name="ps_pool", bufs=1, space="PSUM") as ps_pool,
            ):
                for li in range(L):
                    h_in = hA if li % 2 == 0 else hB
                    h_out = hB if li % 2 == 0 else hA

                    wlw_t = wpool.tile([128, D], F32, tag="wlw")
                    nc.sync.dma_start(out=wlw_t[:], in_=wlw[li])
                    cw0 = wpool.tile([128, D], F32, tag="cw0")
                    nc.sync.dma_start(out=cw0[:], in_=convw[li, 0:128, :])
                    cw1 = wpool.tile([128, D], F32, tag="cw1")
                    nc.sync.dma_start(out=cw1[:], in_=convw[li, 128:256, :])
                    if flags["ln_affine"]:
                        lnsc_t = wpool.tile([128, D], F32, tag="lnsc")
                        nc.sync.dma_start(out=lnsc_t[:], in_=lnsc[li])
                        lnbs_t = wpool.tile([128, D], F32, tag="lnbs")
                        nc.sync.dma_start(out=lnbs_t[:], in_=lnbs[li])
                    if flags["wl_b"]:
                        wlb_t = wpool.tile([128, D], F32, tag="wlb")
                        nc.sync.dma_start(out=wlb_t[:], in_=wlb[li])
                    if flags["conv_b"]:
                        convb_t = wpool.tile([128, D], F32, tag="convb")
                        nc.sync.dma_start(out=convb_t[:], in_=convb[li])

                    # ---- LayerNorm: h_in -> y_c ----
                    for (gb, nb) in groups:
                        rows = slice(gb * 128, (gb + nb) * 128)
                        h_ap = h_in[rows, :].rearrange("(j p) d -> p j d", p=128)
                        ht = lp.tile([128, nb, D], F32, tag="ht")
                        nc.sync.dma_start(out=ht[:], in_=h_ap)
                        mu = lp.tile([128, 4], F32, tag="mu")
                        nc.vector.tensor_reduce(
                            out=mu[:, :nb], in_=ht[:], axis=mybir.AxisListType.X,
                            op=ALU.add,
                        )
                        mus = lp.tile([128, 4], F32, tag="mus")
                        nc.scalar.mul(mus[:, :nb], mu[:, :nb], 1.0 / D)
                        hc = lp.tile([128, nb, D], F32, tag="hc")
                        nc.vector.tensor_tensor(
                            out=hc[:], in0=ht[:],
                            in1=mus[:, :nb, None].to_broadcast([128, nb, D]),
                            op=ALU.subtract,
                        )
                        sq = lp.tile([128, nb, D], F32, tag="sq")
                        nc.scalar.square(sq[:], hc[:])
                        vs = lp.tile([128, 4], F32, tag="vs")
                        nc.vector.tensor_reduce(
                            out=vs[:, :nb], in_=sq[:], axis=mybir.AxisListType.X,
                            op=ALU.add,
                        )
                        sd = lp.tile([128, 4], F32, tag="sd")
                        nc.scalar.activation(
                            sd[:, :nb], vs[:, :nb], ACTF.Sqrt,
                            bias=LN_EPS, scale=1.0 / D,
                        )
                        rs = lp.tile([128, 4], F32, tag="rs")
                        nc.vector.reciprocal(rs[:, :nb], sd[:, :nb])
                        yt = lp.tile([128, nb, D], F32, tag="yt")
                        nc.vector.tensor_tensor(
                            out=yt[:], in0=hc[:],
                            in1=rs[:, :nb, None].to_broadcast([128, nb, D]),
                            op=ALU.mult,
                        )
                        if flags["ln_affine"]:
                            nc.vector.tensor_tensor(
                                out=yt[:], in0=yt[:],
                                in1=lnsc_t[:, None, :].to_broadcast([128, nb, D]),
                                op=ALU.mult,
                            )
                            nc.vector.tensor_tensor(
                                out=yt[:], in0=yt[:],
                                in1=lnbs_t[:, None, :].to_broadcast([128, nb, D]),
                                op=ALU.add,
                            )
                        y_ap = y_c[rows, :].rearrange("(j p) d -> p j d", p=128)
                        nc.sync.dma_start(out=y_ap, in_=yt[:])

                    # ---- AllGather y ----
                    y_full = y_fulls[li]
                    nc.gpsimd.collective_compute(
                        "AllGather", ALU.bypass,
                        replica_groups=[list(range(NC))],
                        ins=[y_c[:].opt()], outs=[y_full[:].opt()],
                    )

                    # ---- edges + conv + pool ----
                    ppool = ps_pool.tile([128, D], F32, tag="ppool")
                    cur_g = -1
                    esrc_t = emeta_t = None
                    for b in range(NBLK):
                        nd = ps_nd.tile([128, 2 * D], F32, tag="nd")
                        t0, t1 = int(tstart[b]), int(tstart[b + 1])
                        for t in range(t0, t1):
                            g, j = t // 4, t % 4
                            if g != cur_g:
                                esrc_t = xp.tile([128, 4], I32, tag="esrc")
                                nc.sync.dma_start(out=esrc_t[:], in_=esrc[g])
                                emeta_t = xp.tile([128, 8], F32, tag="emeta")
                                nc.sync.dma_start(out=emeta_t[:], in_=emeta[g])
                                cur_g = g
                            ysrc = xp.tile([128, D], F32, tag="ysrc")
                            nc.gpsimd.indirect_dma_start(
                                out=ysrc[:], out_offset=None, in_=y_full[:],
                                in_offset=bass.IndirectOffsetOnAxis(
                                    ap=esrc_t[:, j:j + 1], axis=0
                                ),
                            )
                            st = xp.tile([128, 128], F32, tag="st")
                            nc.vector.tensor_tensor(
                                out=st[:],
                                in0=emeta_t[:, j:j + 1].to_broadcast([128, 128]),
                                in1=colidx_t[:],
                                op=ALU.is_equal,
                            )
                            pre = xp.tile([128, D], F32, tag="pre")
                            nc.vector.scalar_tensor_tensor(
                                out=pre[:], in0=wlw_t[:],
                                scalar=emeta_t[:, 4 + j:5 + j], in1=ysrc[:],
                                op0=ALU.mult, op1=ALU.add,
                            )
                            if flags["wl_b"]:
                                nc.vector.tensor_tensor(
                                    out=pre[:], in0=pre[:], in1=wlb_t[:],
                                    op=ALU.add,
                                )
                            msg = xp.tile([128, D], F32, tag="msg")
                            nc.scalar.activation(msg[:], pre[:], ACTF.Relu)
                            ev = xp.tile([128, 2 * D], F32, tag="ev")
                            nc.scalar.activation(ev[:, :D], msg[:], ACTF.Exp)
                            nc.vector.tensor_tensor(
                                out=ev[:, D:], in0=msg[:], in1=ev[:, :D],
                                op=ALU.mult,
                            )
                            nc.tensor.matmul(
                                out=nd[:], lhsT=st[:], rhs=ev[:],
                                start=(t == t0), stop=(t == t1 - 1),
                            )
                        # block post: softmax-agg + residual + conv + pool
                        td = bp.tile([128, D], F32, tag="td")
                        nc.vector.tensor_scalar_max(td[:], nd[:, :D], 1e-16)
                        rec = bp.tile([128, D], F32, tag="rec")
                        nc.vector.reciprocal(rec[:], td[:])
                        yb = bp.tile([128, D], F32, tag="yb")
                        nc.sync.dma_start(
                            out=yb[:], in_=y_c[b * 128:(b + 1) * 128, :]
                        )
                        xv = bp.tile([128, D], F32, tag="xv")
                        nc.vector.tensor_tensor(
                            out=xv[:], in0=nd[:, D:], in1=rec[:], op=ALU.mult
                        )
                        nc.vector.tensor_tensor(
                            out=xv[:], in0=xv[:], in1=yb[:], op=ALU.add
                        )
                        pxt = ps_xt.tile([128, D], F32, tag="pxt")
                        nc.tensor.transpose(
                            out=pxt[:, 0:128], in_=xv[:, 0:128], identity=ident_t[:]
                        )
                        nc.tensor.transpose(
                            out=pxt[:, 128:256], in_=xv[:, 128:256],
                            identity=ident_t[:],
                        )
                        xts = bp.tile([128, D], F32, tag="xts")
                        nc.vector.tensor_copy(out=xts[:], in_=pxt[:])
                        ph = ps_h.tile([128, D], F32, tag="ph")
                        nc.tensor.matmul(
                            out=ph[:], lhsT=xts[:, 0:128], rhs=cw0[:],
                            start=True, stop=False,
                        )
                        nc.tensor.matmul(
                            out=ph[:], lhsT=xts[:, 128:256], rhs=cw1[:],
                            start=False, stop=True,
                        )
                        if flags["conv_b"]:
                            nc.vector.tensor_tensor(
                                out=ph[:], in0=ph[:], in1=convb_t[:], op=ALU.add
                            )
                        hb = bp.tile([128, D], F32, tag="hb")
                        nc.scalar.activation(hb[:], ph[:], ACTF.Relu)
                        nc.sync.dma_start(
                            out=h_out[b * 128:(b + 1) * 128, :], in_=hb[:]
                        )
                        blt = bp.tile([128, 1], F32, tag="blt")
                        nc.sync.dma_start(out=blt[:], in_=batchl[b])
                        ind = bp.tile([128, 128], F32, tag="ind")
                        nc.vector.tensor_tensor(
                            out=ind[:],
                            in0=blt[:, :1].to_broadcast([128, 128]),
                            in1=colidx_t[:],
                            op=ALU.is_equal,
                        )
                        nc.tensor.matmul(
                            out=ppool[:], lhsT=ind[:], rhs=hb[:],
                            start=(b == 0), stop=(b == NBLK - 1),
                        )
                    # pool -> z_all
                    zp = bp.tile([128, D], F32, tag="zp")
                    nc.vector.tensor_copy(out=zp[:], in_=ppool[:])
                    zrt = bp.tile([128, 1], I32, tag="zrt")
                    nc.sync.dma_start(out=zrt[:], in_=zrow[li])
                    nc.gpsimd.indirect_dma_start(
                        out=z_all[:],
                        out_offset=bass.IndirectOffsetOnAxis(ap=zrt[:, :1], axis=0),
                        in_=zp[:], in_offset=None,
                    )

            # ---------- AllReduce z ----------
            nc.gpsimd.collective_compute(
                "AllReduce", ALU.add,
                replica_groups=[list(range(NC))],
                ins=[z_all[:].opt()], outs=[z_red[:].opt()],
            )

            # ---------- readout MLP (replicated) ----------
            with (
                tc.tile_pool(name="row", bufs=1) as rw,
                tc.tile_pool(name="ro", bufs=2) as ro,
                tc.tile_pool(name="ps_a", bufs=1, space="PSUM") as psa,
                tc.tile_pool(name="ps_b", bufs=1, space="PSUM") as psb,
                tc.tile_pool(name="ps_t", bufs=2, space="PSUM") as pst,
                tc.tile_pool(name="ps_o", bufs=1, space="PSUM") as pso,
            ):
                w0t = []
                for f in range(12):
                    w = rw.tile([128, 768], F32, tag=f"w0_{f}")
                    nc.sync.dma_start(out=w[:], in_=row0[f * 128:(f + 1) * 128, :])
                    w0t.append(w)
                w1t = []
                for f in range(6):
                    w = rw.tile([128, 384], F32, tag=f"w1_{f}")
                    nc.sync.dma_start(out=w[:], in_=row1[f * 128:(f + 1) * 128, :])
                    w1t.append(w)
                w2t = []
                for f in range(3):
                    w = rw.tile([128, 192], F32, tag=f"w2_{f}")
                    nc.sync.dma_start(out=w[:], in_=row2[f * 128:(f + 1) * 128, :])
                    w2t.append(w)
                w3a = rw.tile([128, 1], F32, tag="w3a")
                nc.sync.dma_start(out=w3a[:], in_=row3[0:128, :])
                w3b = rw.tile([64, 1], F32, tag="w3b")
                nc.sync.dma_start(out=w3b[:], in_=row3[128:192, :])
                robt = []
                if flags["ro_b"]:
                    for i, n in enumerate([768, 384, 192, 1]):
                        w = rw.tile([128, n], F32, tag=f"rob{i}")
                        nc.sync.dma_start(out=w[:], in_=robs[i][:])
                        robt.append(w)

                def transpose_chunk(src_ap, kdim):
                    """src_ap: [128, kdim] SBUF -> returns [kdim,128] SBUF tile."""
                    pt = pst.tile([128, 128], F32, tag="pt")
                    nc.tensor.transpose(
                        out=pt[:kdim, :], in_=src_ap, identity=ident_t[:]
                    )
                    ct = ro.tile([128, 128], F32, tag="ct")
                    nc.vector.tensor_copy(out=ct[:kdim, :], in_=pt[:kdim, :])
                    return ct

                for gb in range(4):
                    pA = psa.tile([128, 512], F32, tag="pA")
                    pB = psb.tile([128, 256], F32, tag="pB")
                    for f in range(12):
                        li, half = f // 2, f % 2
                        zc = ro.tile([128, 128], F32, tag="zc")
                        nc.sync.dma_start(
                            out=zc[:],
                            in_=z_red[
                                512 * li + 128 * gb: 512 * li + 128 * (gb + 1),
                                128 * half: 128 * (half + 1),
                            ],
                        )
                        zt = transpose_chunk(zc[:], 128)
                        nc.tensor.matmul(
                            out=pA[:], lhsT=zt[:], rhs=w0t[f][:, 0:512],
                            start=(f == 0), stop=(f == 11),
                        )
                        nc.tensor.matmul(
                            out=pB[:], lhsT=zt[:], rhs=w0t[f][:, 512:768],
                            start=(f == 0), stop=(f == 11),
                        )
                    z1 = ro.tile([128, 768], F32, tag="z1")
                    if flags["ro_b"]:
                        nc.vector.tensor_tensor(
                            out=pA[:], in0=pA[:], in1=robt[0][:, 0:512], op=ALU.add
                        )
                        nc.vector.tensor_tensor(
                            out=pB[:], in0=pB[:], in1=robt[0][:, 512:768], op=ALU.add
                        )
                    nc.scalar.activation(z1[:, 0:512], pA[:], ACTF.Gelu)
                    nc.scalar.activation(z1[:, 512:768], pB[:], ACTF.Gelu)

                    p2 = psa.tile([128, 384], F32, tag="p2")
                    for f in range(6):
                        zt = transpose_chunk(z1[:, 128 * f:128 * (f + 1)], 128)
                        nc.tensor.matmul(
                            out=p2[:], lhsT=zt[:], rhs=w1t[f][:],
                            start=(f == 0), stop=(f == 5),
                        )
                    if flags["ro_b"]:
                        nc.vector.tensor_tensor(
                            out=p2[:], in0=p2[:], in1=robt[1][:], op=ALU.add
                        )
                    z2 = ro.tile([128, 384], F32, tag="z2")
                    nc.scalar.activation(z2[:], p2[:], ACTF.Gelu)

                    p3 = psb.tile([128, 192], F32, tag="p3")
                    for f in range(3):
                        zt = transpose_chunk(z2[:, 128 * f:128 * (f + 1)], 128)
                        nc.tensor.matmul(
                            out=p3[:], lhsT=zt[:], rhs=w2t[f][:],
                            start=(f == 0), stop=(f == 2),
                        )
                    if flags["ro_b"]:
                        nc.vector.tensor_tensor(
                            out=p3[:], in0=p3[:], in1=robt[2][:], op=ALU.add
                        )
                    z3 = ro.tile([128, 192], F32, tag="z3")
                    nc.scalar.activation(z3[:], p3[:], ACTF.Gelu)

                    po = pso.tile([128, 1], F32, tag="po")
                    zt = transpose_chunk(z3[:, 0:128], 128)
                    nc.tensor.matmul(
                        out=po[:], lhsT=zt[:], rhs=w3a[:],
                        start=True, stop=False,
                    )
                    zt = transpose_chunk(z3[:, 128:192], 64)
                    nc.tensor.matmul(
                        out=po[:], lhsT=zt[:64, :], rhs=w3b[:],
                        start=False, stop=True,
                    )
                    oc = ro.tile([128, 1], F32, tag="oc")
                    if flags["ro_b"]:
                        nc.vector.tensor_tensor(
                            out=po[:], in0=po[:], in1=robt[3][:], op=ALU.add
                        )
                    nc.vector.tensor_copy(out=oc[:], in_=po[:])
                    nc.sync.dma_start(
                        out=out[128 * gb:128 * (gb + 1), :], in_=oc[:]
                    )

    nc.compile()
    return nc


# ----------------------------------------------------------------------------
# entry point
# ----------------------------------------------------------------------------

def kernel(**inputs):
    in_maps, tiles_b, block_of_tile, T, T4, flags = _prep(inputs)
    key = (tuple(tiles_b.tolist()), tuple(sorted(flags.items())))
    if key not in _prog_cache:
        _prog_cache[key] = _build(tiles_b, block_of_tile, T, T4, flags)
    nc = _prog_cache[key]

    kwargs = {}
    if TRACE:
        kwargs = dict(trace=True, trace_cores=TRACE_CORES)
    res = run_bass_kernel_spmd(nc, in_maps, list(range(NC)), **kwargs)
    LAST_RESULT["exec_time_ns"] = getattr(res, "exec_time_ns", None)
    LAST_RESULT["res"] = res
    return np.asarray(res.results[0]["out"], dtype=np.float32)


# revision 22
# speedup vs baseline: 1.1293x; 1.1293x over previous
"""Trainium2 Bass kernel for GCNNetwork (GENConv message passing, L=6).

Strategy (graph-data parallel over 8 NeuronCores):
 - Nodes sharded contiguously: core c owns rows [7552c, 7552c+7552) (N padded
   60000 -> 60416). Edges assigned to the core owning their dst, sorted by dst.
 - Per layer: LayerNorm on the local shard -> AllGather y across cores (y_full)
   -> per 128-edge tile: indirect-DMA gather y[src], compute msg/w=exp(msg)/
   msg*w, segment-reduce to the 128-node block via an indicator matmul
   accumulated in PSUM ([denom | numer]) -> softmax-aggregate, residual,
   conv matmul, relu -> pooling via batch-indicator matmul accumulated in PSUM.
 - Pool windows (128 graphs/core) are indirect-scattered into a global
   [3072+, 256] z buffer, AllReduced, and the readout MLP is computed
   replicated on every core.

All weights are replicated. Biases/LN affine params are applied only when
nonzero/non-one (they are zeros/ones for this model's init).
"""
import sys
import numpy as np

for _p in ("/opt/trn_rl_repo", "/root/.axon_site/_ro/trn_rl_repo"):
    if _p not in sys.path:
        sys.path.append(_p)

import concourse.bass as bass
import concourse.bacc as bacc
import concourse.mybir as mybir
import concourse.tile as tile
from concourse.bass_utils import run_bass_kernel_spmd

F32 = mybir.dt.float32
I32 = mybir.dt.int32
ALU = mybir.AluOpType
ACTF = mybir.ActivationFunctionType

N, E, B, D, L = 60000, 120000, 512, 256, 6
NTYPES = 25
LN_EPS = 1e-5
NC = 8
NPC = N // NC             # 7500 real nodes per core
NBLK = 61                 # 128-slot node blocks per core (bin-packed)
NSHARD = NBLK * 128       # 7808 slots per core
EDGE_CAP = 256            # target in-edges per block (2 tiles)
ZROWS = 3200              # z buffer rows (L*512 = 3072 used, row 3072 = dump)
ZDUMP = 3072

# module-level knobs (test.py pokes these; harness uses defaults)
TRACE = False
TRACE_CORES = None
LAST_RESULT = {}

_prog_cache = {}


def _ceil_div(a, b):
    return (a + b - 1) // b


# ----------------------------------------------------------------------------
# host-side preprocessing
# ----------------------------------------------------------------------------

def _prep(inputs):
    x = np.asarray(inputs["x"]).astype(np.int32).reshape(-1)
    ei = np.asarray(inputs["edge_index"]).astype(np.int64)
    ea = np.asarray(inputs["edge_attr"]).astype(np.float32).reshape(-1)
    batch = np.asarray(inputs["batch"]).astype(np.int64).reshape(-1)
    src_all, dst_all = ei[0], ei[1]

    # ---- pass 1: per-core node permutation (bin-pack by in-degree) ----
    # Nodes are re-ordered into "slots" so that each 128-slot block has a
    # near-equal number of in-edges (snake assignment by degree). Device
    # code only ever sees slot order; all index arrays are remapped here.
    glob_slot = np.zeros(N, dtype=np.int64)        # node id -> global slot row
    slot_node = []                                 # per core: slot -> node id (-1 pad)
    bin_cnts = np.zeros((NC, NBLK), dtype=np.int64)
    for c in range(NC):
        lo, hi = c * NPC, (c + 1) * NPC
        deg = np.bincount(dst_all[(dst_all >= lo) & (dst_all < hi)] - lo,
                          minlength=NPC)
        order = np.argsort(-deg, kind="stable")    # local ids, degree desc
        i = np.arange(NPC)
        chunk, pos = i // NBLK, i % NBLK
        bins = np.where(chunk % 2 == 0, pos, NBLK - 1 - pos)
        # rank within bin = chunk index (each bin gets <=123 nodes)
        cnt = np.zeros(NBLK, dtype=np.int64)
        for b in range(NBLK):
            cnt[b] = deg[order[bins == b]].sum()
        # sort bins by edge count desc so big blocks align across cores
        border = np.argsort(-cnt, kind="stable")
        bin_rank = np.empty(NBLK, dtype=np.int64)
        bin_rank[border] = np.arange(NBLK)
        new_bin = bin_rank[bins]                   # sorted-bin index per order-pos
        bin_cnts[c] = cnt[border]
        sn = np.full(NSHARD, -1, dtype=np.int64)
        sl = new_bin * 128 + chunk                 # slot per order-position
        sn[sl] = order + lo
        slot_node.append(sn)
        loc_slot = np.empty(NPC, dtype=np.int64)
        loc_slot[order] = sl
        glob_slot[lo:hi] = c * NSHARD + loc_slot

    tiles_b = np.maximum(1, _ceil_div(bin_cnts.max(axis=0), 128)).astype(np.int64)
    tile_start = np.concatenate([[0], np.cumsum(tiles_b)])
    T = int(tile_start[-1])
    T4 = _ceil_div(T, 4)
    TT = T4 * 4
    block_of_tile = np.repeat(np.arange(NBLK), tiles_b)

    # ---- pass 2: per-core edge arrays / node arrays in slot order ----
    in_maps = []
    dst_slot_all = glob_slot[dst_all]              # global slot of dst
    for c in range(NC):
        sel = (dst_slot_all >= c * NSHARD) & (dst_slot_all < (c + 1) * NSHARD)
        ds = dst_slot_all[sel] - c * NSHARD        # local slot of dst
        s = src_all[sel]
        a = ea[sel]
        blk = ds >> 7
        o = np.argsort(blk, kind="stable")
        ds, s, a, blk = ds[o], s[o], a[o], blk[o]
        cnts = np.bincount(blk, minlength=NBLK)
        assert np.all(cnts == bin_cnts[c])
        ne = len(ds)
        bstart = np.concatenate([[0], np.cumsum(cnts)])
        rank = np.arange(ne) - np.repeat(bstart[:-1], cnts)
        slot = tile_start[blk] * 128 + rank
        esrc = np.zeros(TT * 128, dtype=np.int32)
        dstl = np.full(TT * 128, -1.0, dtype=np.float32)
        eav = np.zeros(TT * 128, dtype=np.float32)
        esrc[slot] = glob_slot[s]                  # gather rows in slot space
        dstl[slot] = (ds - (blk << 7)).astype(np.float32)
        eav[slot] = a
        esrc_pm = esrc.reshape(TT, 128).T.copy()
        emeta_pm = np.concatenate(
            [dstl.reshape(TT, 128).T, eav.reshape(TT, 128).T], axis=1
        ).astype(np.float32).copy()

        # ---- node arrays in slot order ----
        sn = slot_node[c]
        valid = sn >= 0
        g0 = int(batch[c * NPC])
        bl = np.full(NSHARD, -1.0, dtype=np.float32)
        bl[valid] = (batch[sn[valid]] - g0).astype(np.float32)
        assert bl.max() < 128, "graph window exceeds 128 per core"
        batchl_pm = bl.reshape(NBLK, 128).T.copy()           # [128, NBLK]
        xid = np.zeros(NSHARD, dtype=np.int32)
        xid[valid] = x[sn[valid]]
        xid_pm = xid.reshape(NBLK, 128).T.copy()             # [128, NBLK]
        zrow_pm = np.zeros((128, L), dtype=np.int32)
        g = g0 + np.arange(128)
        for i in range(L):
            zrow_pm[:, i] = np.where(g < B, 512 * i + g, ZDUMP)

        in_maps.append(
            dict(esrc=esrc_pm, emeta=emeta_pm, batchl=batchl_pm,
                 xid=xid_pm, zrow=zrow_pm)
        )

    # ---- shared weights ----
    wl_w = np.asarray(inputs["wl_w"]).astype(np.float32)     # [L,1,D]
    conv_w = np.asarray(inputs["conv_w"]).astype(np.float32)  # [L,D,D]
    node_emb = np.asarray(inputs["node_emb"]).astype(np.float32)
    ln_scale = np.asarray(inputs["ln_scale"]).astype(np.float32)
    ln_bias = np.asarray(inputs["ln_bias"]).astype(np.float32)
    wl_b = np.asarray(inputs["wl_b"]).astype(np.float32)
    conv_b = np.asarray(inputs["conv_b"]).astype(np.float32)
    ro_w = [np.asarray(inputs[f"ro_w{i}"]).astype(np.float32) for i in range(4)]
    ro_b = [np.asarray(inputs[f"ro_b{i}"]).astype(np.float32) for i in range(4)]

    flags = dict(
        ln_affine=not (np.all(ln_scale == 1.0) and np.all(ln_bias == 0.0)),
        wl_b=bool(np.any(wl_b != 0.0)),
        conv_b=bool(np.any(conv_b != 0.0)),
        ro_b=any(np.any(b != 0.0) for b in ro_b),
    )

    shared = dict(
        wlw=np.repeat(wl_w.reshape(L, 1, D), 128, axis=1).copy(),
        convw=conv_w,
        nemb=node_emb,
        colidx=np.tile(np.arange(128, dtype=np.float32), (128, 1)).copy(),
        ident=np.eye(128, dtype=np.float32),
        row0=ro_w[0], row1=ro_w[1], row2=ro_w[2], row3=ro_w[3],
    )
    if flags["ln_affine"]:
        shared["lnsc"] = np.repeat(ln_scale.reshape(L, 1, D), 128, axis=1).copy()
        shared["lnbs"] = np.repeat(ln_bias.reshape(L, 1, D), 128, axis=1).copy()
    if flags["wl_b"]:
        shared["wlb"] = np.repeat(wl_b.reshape(L, 1, D), 128, axis=1).copy()
    if flags["conv_b"]:
        shared["convb"] = np.repeat(conv_b.reshape(L, 1, D), 128, axis=1).copy()
    if flags["ro_b"]:
        for i, b in enumerate(ro_b):
            shared[f"rob{i}"] = np.repeat(b.reshape(1, -1), 128, axis=0).copy()

    for m in in_maps:
        m.update(shared)
    return in_maps, tiles_b, block_of_tile, T, T4, flags


# ----------------------------------------------------------------------------
# device program
# ----------------------------------------------------------------------------

def _build(tiles_b, block_of_tile, T, T4, flags):
    nc = bacc.Bacc("TRN2", target_bir_lowering=False, debug=False, num_devices=NC)

    # register LN epsilon as a const AP (activation float biases need one)
    _eps_t = nc.alloc_sbuf_tensor(f"const-float32-lneps", [128, 1], F32)
    nc.gpsimd.memset(_eps_t.ap(), LN_EPS)
    nc.const_aps.aps[(F32, LN_EPS)] = _eps_t.ap()
    nc.all_engine_barrier()

    # inputs
    TT = T4 * 4
    esrc = nc.dram_tensor("esrc", [128, TT], I32, kind="ExternalInput")
    emeta = nc.dram_tensor("emeta", [128, 2 * TT], F32, kind="ExternalInput")
    batchl = nc.dram_tensor("batchl", [128, NBLK], F32, kind="ExternalInput")
    xid = nc.dram_tensor("xid", [128, NBLK], I32, kind="ExternalInput")
    zrow = nc.dram_tensor("zrow", [128, L], I32, kind="ExternalInput")
    wlw = nc.dram_tensor("wlw", [L, 128, D], F32, kind="ExternalInput")
    convw = nc.dram_tensor("convw", [L, D, D], F32, kind="ExternalInput")
    nemb = nc.dram_tensor("nemb", [NTYPES, D], F32, kind="ExternalInput")
    colidx = nc.dram_tensor("colidx", [128, 128], F32, kind="ExternalInput")
    ident = nc.dram_tensor("ident", [128, 128], F32, kind="ExternalInput")
    row0 = nc.dram_tensor("row0", [6 * D, 768], F32, kind="ExternalInput")
    row1 = nc.dram_tensor("row1", [768, 384], F32, kind="ExternalInput")
    row2 = nc.dram_tensor("row2", [384, 192], F32, kind="ExternalInput")
    row3 = nc.dram_tensor("row3", [192, 1], F32, kind="ExternalInput")
    lnsc = lnbs = wlb = convb = None
    if flags["ln_affine"]:
        lnsc = nc.dram_tensor("lnsc", [L, 128, D], F32, kind="ExternalInput")
        lnbs = nc.dram_tensor("lnbs", [L, 128, D], F32, kind="ExternalInput")
    if flags["wl_b"]:
        wlb = nc.dram_tensor("wlb", [L, 128, D], F32, kind="ExternalInput")
    if flags["conv_b"]:
        convb = nc.dram_tensor("convb", [L, 128, D], F32, kind="ExternalInput")
    robs = None
    if flags["ro_b"]:
        robs = [
            nc.dram_tensor(f"rob{i}", [128, n], F32, kind="ExternalInput")
            for i, n in enumerate([768, 384, 192, 1])
        ]

    out = nc.dram_tensor("out", [B, 1], F32, kind="ExternalOutput")

    # LN groups of up to 4 blocks
    groups = []
    b0 = 0
    while b0 < NBLK:
        nb = min(4, NBLK - b0)
        groups.append((b0, nb))
        b0 += nb

    # tile ranges per block
    tstart = np.concatenate([[0], np.cumsum(tiles_b)]).astype(int)

    with tile.TileContext(nc) as tc:
        with (
            tc.tile_pool(name="dram", bufs=1, space="DRAM") as dram,
            tc.tile_pool(name="consts", bufs=1) as cpool,
            tc.tile_pool(name="lweights", bufs=2) as wpool,
        ):
            hA = dram.tile([NSHARD, D], F32, tag="hA")
            hB = dram.tile([NSHARD, D], F32, tag="hB")
            y_c = dram.tile([NSHARD, D], F32, tag="y_c")
            y_fulls = [
                dram.tile([NC * NSHARD, D], F32, tag=f"y_full_{i}",
                          name=f"y_full_{i}", addr_space="Shared")
                for i in range(L)
            ]
            z_all = dram.tile([ZROWS, D], F32, tag="z_all")
            z_red = dram.tile([ZROWS, D], F32, tag="z_red", addr_space="Shared")

            colidx_t = cpool.tile([128, 128], F32, tag="colidx")
            nc.sync.dma_start(out=colidx_t[:], in_=colidx[:])
            ident_t = cpool.tile([128, 128], F32, tag="ident")
            nc.sync.dma_start(out=ident_t[:], in_=ident[:])
            # whole-kernel metadata, loaded once (partition-major)
            esrc_sb = cpool.tile([128, TT], I32, tag="esrc_sb")
            nc.sync.dma_start(out=esrc_sb[:], in_=esrc[:])
            emeta_sb = cpool.tile([128, 2 * TT], F32, tag="emeta_sb")
            nc.sync.dma_start(out=emeta_sb[:], in_=emeta[:])
            batchl_sb = cpool.tile([128, NBLK], F32, tag="batchl_sb")
            nc.sync.dma_start(out=batchl_sb[:], in_=batchl[:])
            xid_sb = cpool.tile([128, NBLK], I32, tag="xid_sb")
            nc.sync.dma_start(out=xid_sb[:], in_=xid[:])
            zrow_sb = cpool.tile([128, L], I32, tag="zrow_sb")
            nc.sync.dma_start(out=zrow_sb[:], in_=zrow[:])
            # pooling indicators, built once (gpsimd; reused across layers)
            ind_ts = []
            for b in range(NBLK):
                ind_b = cpool.tile([128, 128], F32, tag=f"ind{b}",
                                   name=f"ind{b}")
                nc.vector.tensor_tensor(
                    out=ind_b[:],
                    in0=batchl_sb[:, b:b + 1].to_broadcast([128, 128]),
                    in1=colidx_t[:],
                    op=ALU.is_equal,
                )
                ind_ts.append(ind_b)

            # ---------- zero z_all ----------
            with tc.tile_pool(name="zz", bufs=1) as zz:
                zt = zz.tile([128, D], F32)
                nc.vector.memset(zt[:], 0.0)
                for k in range(ZROWS // 128):
                    nc.sync.dma_start(
                        out=z_all[k * 128:(k + 1) * 128, :], in_=zt[:]
                    )

            # ---------- embedding ----------
            with tc.tile_pool(name="emb", bufs=4) as ep:
                for b in range(NBLK):
                    et = ep.tile([128, D], F32, tag="et")
                    nc.gpsimd.indirect_dma_start(
                        out=et[:], out_offset=None, in_=nemb[:],
                        in_offset=bass.IndirectOffsetOnAxis(
                            ap=xid_sb[:, b:b + 1], axis=0
                        ),
                    )
                    nc.sync.dma_start(
                        out=hA[b * 128:(b + 1) * 128, :], in_=et[:]
                    )

            # ---------- layers ----------
            with (
                tc.tile_pool(name="ln", bufs=2) as lp,
                tc.tile_pool(name="edge", bufs=4) as xp,
                tc.tile_pool(name="blk", bufs=3) as bp,
                tc.tile_pool(name="ps_nd", bufs=3, space="PSUM") as ps_nd,
                tc.tile_pool(name="ps_xt", bufs=2, space="PSUM") as ps_xt,
                tc.tile_pool(name="ps_h", bufs=1, space="PSUM") as ps_h,
                tc.tile_pool(name="ps_pool", bufs=1, space="PSUM") as ps_pool,
            ):
                for li in range(L):
                    h_in = hA if li % 2 == 0 else hB
                    h_out = hB if li % 2 == 0 else hA

                    wlw_t = wpool.tile([128, D], F32, tag="wlw")
                    nc.sync.dma_start(out=wlw_t[:], in_=wlw[li])
                    cw0 = wpool.tile([128, D], F32, tag="cw0")
                    nc.sync.dma_start(out=cw0[:], in_=convw[li, 0:128, :])
                    cw1 = wpool.tile([128, D], F32, tag="cw1")
                    nc.sync.dma_start(out=cw1[:], in_=convw[li, 128:256, :])
                    if flags["ln_affine"]:
                        lnsc_t = wpool.tile([128, D], F32, tag="lnsc")
                        nc.sync.dma_start(out=lnsc_t[:], in_=lnsc[li])
                        lnbs_t = wpool.tile([128, D], F32, tag="lnbs")
                        nc.sync.dma_start(out=lnbs_t[:], in_=lnbs[li])
                    if flags["wl_b"]:
                        wlb_t = wpool.tile([128, D], F32, tag="wlb")
                        nc.sync.dma_start(out=wlb_t[:], in_=wlb[li])
                    if flags["conv_b"]:
                        convb_t = wpool.tile([128, D], F32, tag="convb")
                        nc.sync.dma_start(out=convb_t[:], in_=convb[li])

                    # ---- LayerNorm: h_in -> y_c ----
                    for (gb, nb) in groups:
                        rows = slice(gb * 128, (gb + nb) * 128)
                        h_ap = h_in[rows, :].rearrange("(j p) d -> p j d", p=128)
                        ht = lp.tile([128, nb, D], F32, tag="ht")
                        nc.sync.dma_start(out=ht[:], in_=h_ap)
                        mu = lp.tile([128, 4], F32, tag="mu")
                        nc.vector.tensor_reduce(
                            out=mu[:, :nb], in_=ht[:], axis=mybir.AxisListType.X,
                            op=ALU.add,
                        )
                        mus = lp.tile([128, 4], F32, tag="mus")
                        nc.scalar.mul(mus[:, :nb], mu[:, :nb], 1.0 / D)
                        hc = lp.tile([128, nb, D], F32, tag="hc")
                        nc.vector.tensor_tensor(
                            out=hc[:], in0=ht[:],
                            in1=mus[:, :nb, None].to_broadcast([128, nb, D]),
                            op=ALU.subtract,
                        )
                        sq = lp.tile([128, nb, D], F32, tag="sq")
                        nc.scalar.square(sq[:], hc[:])
                        vs = lp.tile([128, 4], F32, tag="vs")
                        nc.vector.tensor_reduce(
                            out=vs[:, :nb], in_=sq[:], axis=mybir.AxisListType.X,
                            op=ALU.add,
                        )
                        sd = lp.tile([128, 4], F32, tag="sd")
                        nc.scalar.activation(
                            sd[:, :nb], vs[:, :nb], ACTF.Sqrt,
                            bias=LN_EPS, scale=1.0 / D,
                        )
                        rs = lp.tile([128, 4], F32, tag="rs")
                        nc.vector.reciprocal(rs[:, :nb], sd[:, :nb])
                        yt = lp.tile([128, nb, D], F32, tag="yt")
                        nc.vector.tensor_tensor(
                            out=yt[:], in0=hc[:],
                            in1=rs[:, :nb, None].to_broadcast([128, nb, D]),
                            op=ALU.mult,
                        )
                        if flags["ln_affine"]:
                            nc.vector.tensor_tensor(
                                out=yt[:], in0=yt[:],
                                in1=lnsc_t[:, None, :].to_broadcast([128, nb, D]),
                                op=ALU.mult,
                            )
                            nc.vector.tensor_tensor(
                                out=yt[:], in0=yt[:],
                                in1=lnbs_t[:, None, :].to_broadcast([128, nb, D]),
                                op=ALU.add,
                            )
                        y_ap = y_c[rows, :].rearrange("(j p) d -> p j d", p=128)
                        nc.sync.dma_start(out=y_ap, in_=yt[:])

                    # ---- AllGather y ----
                    y_full = y_fulls[li]
                    nc.gpsimd.collective_compute(
                        "AllGather", ALU.bypass,
                        replica_groups=[list(range(NC))],
                        ins=[y_c[:].opt()], outs=[y_full[:].opt()],
                    )

                    # ---- edges + conv + pool ----
                    ppool = ps_pool.tile([128, D], F32, tag="ppool")
                    for b in range(NBLK):
                        nd = ps_nd.tile([128, 2 * D], F32, tag="nd")
                        t0, t1 = int(tstart[b]), int(tstart[b + 1])
                        for t in range(t0, t1):
                            ysrc = xp.tile([128, D], F32, tag="ysrc")
                            nc.gpsimd.indirect_dma_start(
                                out=ysrc[:], out_offset=None, in_=y_full[:],
                                in_offset=bass.IndirectOffsetOnAxis(
                                    ap=esrc_sb[:, t:t + 1], axis=0
                                ),
                            )
                            st = xp.tile([128, 128], F32, tag="st")
                            nc.vector.tensor_tensor(
                                out=st[:],
                                in0=emeta_sb[:, t:t + 1].to_broadcast([128, 128]),
                                in1=colidx_t[:],
                                op=ALU.is_equal,
                            )
                            pre = xp.tile([128, D], F32, tag="pre")
                            nc.vector.scalar_tensor_tensor(
                                out=pre[:], in0=wlw_t[:],
                                scalar=emeta_sb[:, TT + t:TT + t + 1], in1=ysrc[:],
                                op0=ALU.mult, op1=ALU.add,
                            )
                            if flags["wl_b"]:
                                nc.vector.tensor_tensor(
                                    out=pre[:], in0=pre[:], in1=wlb_t[:],
                                    op=ALU.add,
                                )
                            msg = xp.tile([128, D], F32, tag="msg")
                            nc.scalar.activation(msg[:], pre[:], ACTF.Relu)
                            ev = xp.tile([128, 2 * D], F32, tag="ev")
                            nc.scalar.activation(ev[:, :D], msg[:], ACTF.Exp)
                            nc.vector.tensor_tensor(
                                out=ev[:, D:], in0=msg[:], in1=ev[:, :D],
                                op=ALU.mult,
                            )
                            nc.tensor.matmul(
                                out=nd[:], lhsT=st[:], rhs=ev[:],
                                start=(t == t0), stop=(t == t1 - 1),
                            )
                        # block post: softmax-agg + residual + conv + pool
                        td = bp.tile([128, D], F32, tag="td")
                        nc.vector.tensor_scalar_max(td[:], nd[:, :D], 1e-16)
                        yb = bp.tile([128, D], F32, tag="yb")
                        nc.sync.dma_start(
                            out=yb[:], in_=y_c[b * 128:(b + 1) * 128, :]
                        )
                        rec = bp.tile([128, D], F32, tag="rec")
                        nc.vector.reciprocal(rec[:], td[:])
                        xv = bp.tile([128, D], F32, tag="xv")
                        nc.vector.tensor_tensor(
                            out=xv[:], in0=nd[:, D:], in1=rec[:], op=ALU.mult
                        )
                        nc.vector.tensor_tensor(
                            out=xv[:], in0=xv[:], in1=yb[:], op=ALU.add
                        )
                        pxt = ps_xt.tile([128, D], F32, tag="pxt")
                        nc.tensor.transpose(
                            out=pxt[:, 0:128], in_=xv[:, 0:128], identity=ident_t[:]
                        )
                        nc.tensor.transpose(
                            out=pxt[:, 128:256], in_=xv[:, 128:256],
                            identity=ident_t[:],
                        )
                        xts = bp.tile([128, D], F32, tag="xts")
                        nc.vector.tensor_copy(out=xts[:], in_=pxt[:])
                        ph = ps_h.tile([128, D], F32, tag="ph")
                        nc.tensor.matmul(
                            out=ph[:], lhsT=xts[:, 0:128], rhs=cw0[:],
                            start=True, stop=False,
                        )
                        nc.tensor.matmul(
                            out=ph[:], lhsT=xts[:, 128:256], rhs=cw1[:],
                            start=False, stop=True,
                        )
                        if flags["conv_b"]:
                            nc.vector.tensor_tensor(
                                out=ph[:], in0=ph[:], in1=convb_t[:], op=ALU.add
                            )
                        hb = bp.tile([128, D], F32, tag="hb")
                        nc.scalar.activation(hb[:], ph[:], ACTF.Relu)
                        nc.sync.dma_start(
                            out=h_out[b * 128:(b + 1) * 128, :], in_=hb[:]
                        )
                        nc.tensor.matmul(
                            out=ppool[:], lhsT=ind_ts[b][:], rhs=hb[:],
                            start=(b == 0), stop=(b == NBLK - 1),
                        )
                    # pool -> z_all
                    zp = bp.tile([128, D], F32, tag="zp")
                    nc.vector.tensor_copy(out=zp[:], in_=ppool[:])
                    nc.gpsimd.indirect_dma_start(
                        out=z_all[:],
                        out_offset=bass.IndirectOffsetOnAxis(
                            ap=zrow_sb[:, li:li + 1], axis=0
                        ),
                        in_=zp[:], in_offset=None,
                    )

            # ---------- AllReduce z ----------
            nc.gpsimd.collective_compute(
                "AllReduce", ALU.add,
                replica_groups=[list(range(NC))],
                ins=[z_all[:].opt()], outs=[z_red[:].opt()],
            )

            # ---------- readout MLP (replicated) ----------
            with (
                tc.tile_pool(name="row", bufs=1) as rw,
                tc.tile_pool(name="ro", bufs=2) as ro,
                tc.tile_pool(name="ps_a", bufs=1, space="PSUM") as psa,
                tc.tile_pool(name="ps_b", bufs=1, space="PSUM") as psb,
                tc.tile_pool(name="ps_t", bufs=2, space="PSUM") as pst,
                tc.tile_pool(name="ps_o", bufs=1, space="PSUM") as pso,
            ):
                w0t = []
                for f in range(12):
                    w = rw.tile([128, 768], F32, tag=f"w0_{f}")
                    nc.sync.dma_start(out=w[:], in_=row0[f * 128:(f + 1) * 128, :])
                    w0t.append(w)
                w1t = []
                for f in range(6):
                    w = rw.tile([128, 384], F32, tag=f"w1_{f}")
                    nc.sync.dma_start(out=w[:], in_=row1[f * 128:(f + 1) * 128, :])
                    w1t.append(w)
                w2t = []
                for f in range(3):
                    w = rw.tile([128, 192], F32, tag=f"w2_{f}")
                    nc.sync.dma_start(out=w[:], in_=row2[f * 128:(f + 1) * 128, :])
                    w2t.append(w)
                w3a = rw.tile([128, 1], F32, tag="w3a")
                nc.sync.dma_start(out=w3a[:], in_=row3[0:128, :])
                w3b = rw.tile([64, 1], F32, tag="w3b")
                nc.sync.dma_start(out=w3b[:], in_=row3[128:192, :])
                robt = []
                if flags["ro_b"]:
                    for i, n in enumerate([768, 384, 192, 1]):
                        w = rw.tile([128, n], F32, tag=f"rob{i}")
                        nc.sync.dma_start(out=w[:], in_=robs[i][:])
                        robt.append(w)

                def transpose_chunk(src_ap, kdim):
                    """src_ap: [128, kdim] SBUF -> returns [kdim,128] SBUF tile."""
                    pt = pst.tile([128, 128], F32, tag="pt")
                    nc.tensor.transpose(
                        out=pt[:kdim, :], in_=src_ap, identity=ident_t[:]
                    )
                    ct = ro.tile([128, 128], F32, tag="ct")
                    nc.vector.tensor_copy(out=ct[:kdim, :], in_=pt[:kdim, :])
                    return ct

                for gb in range(4):
                    pA = psa.tile([128, 512], F32, tag="pA")
                    pB = psb.tile([128, 256], F32, tag="pB")
                    for f in range(12):
                        li, half = f // 2, f % 2
                        zc = ro.tile([128, 128], F32, tag="zc")
                        nc.sync.dma_start(
                            out=zc[:],
                            in_=z_red[
                                512 * li + 128 * gb: 512 * li + 128 * (gb + 1),
                                128 * half: 128 * (half + 1),
                            ],
                        )
                        zt = transpose_chunk(zc[:], 128)
                        nc.tensor.matmul(
                            out=pA[:], lhsT=zt[:], rhs=w0t[f][:, 0:512],
                            start=(f == 0), stop=(f == 11),
                        )
                        nc.tensor.matmul(
                            out=pB[:], lhsT=zt[:], rhs=w0t[f][:, 512:768],
                            start=(f == 0), stop=(f == 11),
                        )
                    z1 = ro.tile([128, 768], F32, tag="z1")
                    if flags["ro_b"]:
                        nc.vector.tensor_tensor(
                            out=pA[:], in0=pA[:], in1=robt[0][:, 0:512], op=ALU.add
                        )
                        nc.vector.tensor_tensor(
                            out=pB[:], in0=pB[:], in1=robt[0][:, 512:768], op=ALU.add
                        )
                    nc.scalar.activation(z1[:, 0:512], pA[:], ACTF.Gelu)
                    nc.scalar.activation(z1[:, 512:768], pB[:], ACTF.Gelu)

                    p2 = psa.tile([128, 384], F32, tag="p2")
                    for f in range(6):
                        zt = transpose_chunk(z1[:, 128 * f:128 * (f + 1)], 128)
                        nc.tensor.matmul(
                            out=p2[:], lhsT=zt[:], rhs=w1t[f][:],
                            start=(f == 0), stop=(f == 5),
                        )
                    if flags["ro_b"]:
                        nc.vector.tensor_tensor(
                            out=p2[:], in0=p2[:], in1=robt[1][:], op=ALU.add
                        )
                    z2 = ro.tile([128, 384], F32, tag="z2")
                    nc.scalar.activation(z2[:], p2[:], ACTF.Gelu)

                    p3 = psb.tile([128, 192], F32, tag="p3")
                    for f in range(3):
                        zt = transpose_chunk(z2[:, 128 * f:128 * (f + 1)], 128)
                        nc.tensor.matmul(
                            out=p3[:], lhsT=zt[:], rhs=w2t[f][:],
                            start=(f == 0), stop=(f == 2),
                        )
                    if flags["ro_b"]:
                        nc.vector.tensor_tensor(
                            out=p3[:], in0=p3[:], in1=robt[2][:], op=ALU.add
                        )
                    z3 = ro.tile([128, 192], F32, tag="z3")
                    nc.scalar.activation(z3[:], p3[:], ACTF.Gelu)

                    po = pso.tile([128, 1], F32, tag="po")
                    zt = transpose_chunk(z3[:, 0:128], 128)
                    nc.tensor.matmul(
                        out=po[:], lhsT=zt[:], rhs=w3a[:],
                        start=True, stop=False,
                    )
                    zt = transpose_chunk(z3[:, 128:192], 64)
                    nc.tensor.matmul(
                        out=po[:], lhsT=zt[:64, :], rhs=w3b[:],
                        start=False, stop=True,
                    )
                    oc = ro.tile([128, 1], F32, tag="oc")
                    if flags["ro_b"]:
                        nc.vector.tensor_tensor(
                            out=po[:], in0=po[:], in1=robt[3][:], op=ALU.add
                        )
                    nc.vector.tensor_copy(out=oc[:], in_=po[:])
                    nc.sync.dma_start(
                        out=out[128 * gb:128 * (gb + 1), :], in_=oc[:]
                    )

    nc.compile()
    return nc


# ----------------------------------------------------------------------------
# entry point
# ----------------------------------------------------------------------------

def kernel(**inputs):
    in_maps, tiles_b, block_of_tile, T, T4, flags = _prep(inputs)
    key = (tuple(tiles_b.tolist()), tuple(sorted(flags.items())))
    if key not in _prog_cache:
        _prog_cache[key] = _build(tiles_b, block_of_tile, T, T4, flags)
    nc = _prog_cache[key]

    kwargs = {}
    if TRACE:
        kwargs = dict(trace=True, trace_cores=TRACE_CORES)
    res = run_bass_kernel_spmd(nc, in_maps, list(range(NC)), **kwargs)
    LAST_RESULT["exec_time_ns"] = getattr(res, "exec_time_ns", None)
    LAST_RESULT["res"] = res
    return np.asarray(res.results[0]["out"], dtype=np.float32)


# revision 28
# speedup vs baseline: 1.5044x; 1.3322x over previous
"""Trainium2 Bass kernel for GCNNetwork (GENConv message passing, L=6).

Strategy (graph-data parallel over 8 NeuronCores):
 - Nodes sharded contiguously: core c owns rows [7552c, 7552c+7552) (N padded
   60000 -> 60416). Edges assigned to the core owning their dst, sorted by dst.
 - Per layer: LayerNorm on the local shard -> AllGather y across cores (y_full)
   -> per 128-edge tile: indirect-DMA gather y[src], compute msg/w=exp(msg)/
   msg*w, segment-reduce to the 128-node block via an indicator matmul
   accumulated in PSUM ([denom | numer]) -> softmax-aggregate, residual,
   conv matmul, relu -> pooling via batch-indicator matmul accumulated in PSUM.
 - Pool windows (128 graphs/core) are indirect-scattered into a global
   [3072+, 256] z buffer, AllReduced, and the readout MLP is computed
   replicated on every core.

All weights are replicated. Biases/LN affine params are applied only when
nonzero/non-one (they are zeros/ones for this model's init).
"""
import sys
import numpy as np

for _p in ("/opt/trn_rl_repo", "/root/.axon_site/_ro/trn_rl_repo"):
    if _p not in sys.path:
        sys.path.append(_p)

import concourse.bass as bass
import concourse.bacc as bacc
import concourse.mybir as mybir
import concourse.tile as tile
from concourse.bass_utils import run_bass_kernel_spmd

F32 = mybir.dt.float32
F16 = mybir.dt.float16
I32 = mybir.dt.int32
FP16_Y = True  # AllGather / gather y in fp16 (halves collective + gather bytes)
ALU = mybir.AluOpType
ACTF = mybir.ActivationFunctionType

N, E, B, D, L = 60000, 120000, 512, 256, 6
NTYPES = 25
LN_EPS = 1e-5
NC = 8
NPC = N // NC             # 7500 real nodes per core
NBLK = 61                 # 128-slot node blocks per core (bin-packed)
NSHARD = NBLK * 128       # 7808 slots per core
EDGE_CAP = 256            # target in-edges per block (2 tiles)
ZROWS = 3200              # z buffer rows (L*512 = 3072 used, row 3072 = dump)
ZDUMP = 3072

# module-level knobs (test.py pokes these; harness uses defaults)
TRACE = False
TRACE_CORES = None
LAST_RESULT = {}

_prog_cache = {}


def _ceil_div(a, b):
    return (a + b - 1) // b


# ----------------------------------------------------------------------------
# host-side preprocessing
# ----------------------------------------------------------------------------

def _prep(inputs):
    x = np.asarray(inputs["x"]).astype(np.int32).reshape(-1)
    ei = np.asarray(inputs["edge_index"]).astype(np.int64)
    ea = np.asarray(inputs["edge_attr"]).astype(np.float32).reshape(-1)
    batch = np.asarray(inputs["batch"]).astype(np.int64).reshape(-1)
    src_all, dst_all = ei[0], ei[1]

    # ---- pass 1: per-core node permutation (bin-pack by in-degree) ----
    # Nodes are re-ordered into "slots" so that each 128-slot block has a
    # near-equal number of in-edges (snake assignment by degree). Device
    # code only ever sees slot order; all index arrays are remapped here.
    glob_slot = np.zeros(N, dtype=np.int64)        # node id -> global slot row
    slot_node = []                                 # per core: slot -> node id (-1 pad)
    bin_cnts = np.zeros((NC, NBLK), dtype=np.int64)
    for c in range(NC):
        lo, hi = c * NPC, (c + 1) * NPC
        deg = np.bincount(dst_all[(dst_all >= lo) & (dst_all < hi)] - lo,
                          minlength=NPC)
        order = np.argsort(-deg, kind="stable")    # local ids, degree desc
        i = np.arange(NPC)
        chunk, pos = i // NBLK, i % NBLK
        bins = np.where(chunk % 2 == 0, pos, NBLK - 1 - pos)
        # rank within bin = chunk index (each bin gets <=123 nodes)
        cnt = np.zeros(NBLK, dtype=np.int64)
        for b in range(NBLK):
            cnt[b] = deg[order[bins == b]].sum()
        # sort bins by edge count desc so big blocks align across cores
        border = np.argsort(-cnt, kind="stable")
        bin_rank = np.empty(NBLK, dtype=np.int64)
        bin_rank[border] = np.arange(NBLK)
        new_bin = bin_rank[bins]                   # sorted-bin index per order-pos
        bin_cnts[c] = cnt[border]
        sn = np.full(NSHARD, -1, dtype=np.int64)
        sl = new_bin * 128 + chunk                 # slot per order-position
        sn[sl] = order + lo
        slot_node.append(sn)
        loc_slot = np.empty(NPC, dtype=np.int64)
        loc_slot[order] = sl
        glob_slot[lo:hi] = c * NSHARD + loc_slot

    tiles_b = np.maximum(1, _ceil_div(bin_cnts.max(axis=0), 128)).astype(np.int64)
    tile_start = np.concatenate([[0], np.cumsum(tiles_b)])
    T = int(tile_start[-1])
    T4 = _ceil_div(T, 4)
    TT = T4 * 4
    block_of_tile = np.repeat(np.arange(NBLK), tiles_b)

    # ---- pass 2: per-core edge arrays / node arrays in slot order ----
    in_maps = []
    dst_slot_all = glob_slot[dst_all]              # global slot of dst
    for c in range(NC):
        sel = (dst_slot_all >= c * NSHARD) & (dst_slot_all < (c + 1) * NSHARD)
        ds = dst_slot_all[sel] - c * NSHARD        # local slot of dst
        s = src_all[sel]
        a = ea[sel]
        blk = ds >> 7
        o = np.argsort(blk, kind="stable")
        ds, s, a, blk = ds[o], s[o], a[o], blk[o]
        cnts = np.bincount(blk, minlength=NBLK)
        assert np.all(cnts == bin_cnts[c])
        ne = len(ds)
        bstart = np.concatenate([[0], np.cumsum(cnts)])
        rank = np.arange(ne) - np.repeat(bstart[:-1], cnts)
        slot = tile_start[blk] * 128 + rank
        esrc = np.zeros(TT * 128, dtype=np.int32)
        dstl = np.full(TT * 128, -1.0, dtype=np.float32)
        eav = np.zeros(TT * 128, dtype=np.float32)
        esrc[slot] = glob_slot[s]                  # gather rows in slot space
        dstl[slot] = (ds - (blk << 7)).astype(np.float32)
        eav[slot] = a
        esrc_pm = esrc.reshape(TT, 128).T.copy()
        emeta_pm = np.concatenate(
            [dstl.reshape(TT, 128).T, eav.reshape(TT, 128).T], axis=1
        ).astype(np.float32).copy()

        # ---- node arrays in slot order ----
        sn = slot_node[c]
        valid = sn >= 0
        g0 = int(batch[c * NPC])
        bl = np.full(NSHARD, -1.0, dtype=np.float32)
        bl[valid] = (batch[sn[valid]] - g0).astype(np.float32)
        assert bl.max() < 128, "graph window exceeds 128 per core"
        batchl_pm = bl.reshape(NBLK, 128).T.copy()           # [128, NBLK]
        xid = np.zeros(NSHARD, dtype=np.int32)
        xid[valid] = x[sn[valid]]
        xid_pm = xid.reshape(NBLK, 128).T.copy()             # [128, NBLK]
        zrow_pm = np.zeros((128, L), dtype=np.int32)
        g = g0 + np.arange(128)
        for i in range(L):
            zrow_pm[:, i] = np.where(g < B, 512 * i + g, ZDUMP)

        in_maps.append(
            dict(esrc=esrc_pm, emeta=emeta_pm, batchl=batchl_pm,
                 xid=xid_pm, zrow=zrow_pm)
        )

    # ---- shared weights ----
    wl_w = np.asarray(inputs["wl_w"]).astype(np.float32)     # [L,1,D]
    conv_w = np.asarray(inputs["conv_w"]).astype(np.float32)  # [L,D,D]
    node_emb = np.asarray(inputs["node_emb"]).astype(np.float32)
    ln_scale = np.asarray(inputs["ln_scale"]).astype(np.float32)
    ln_bias = np.asarray(inputs["ln_bias"]).astype(np.float32)
    wl_b = np.asarray(inputs["wl_b"]).astype(np.float32)
    conv_b = np.asarray(inputs["conv_b"]).astype(np.float32)
    ro_w = [np.asarray(inputs[f"ro_w{i}"]).astype(np.float32) for i in range(4)]
    ro_b = [np.asarray(inputs[f"ro_b{i}"]).astype(np.float32) for i in range(4)]

    flags = dict(
        ln_affine=not (np.all(ln_scale == 1.0) and np.all(ln_bias == 0.0)),
        wl_b=bool(np.any(wl_b != 0.0)),
        conv_b=bool(np.any(conv_b != 0.0)),
        ro_b=any(np.any(b != 0.0) for b in ro_b),
    )

    shared = dict(
        wlw=np.repeat(wl_w.reshape(L, 1, D), 128, axis=1).copy(),
        convw=conv_w,
        nemb=node_emb,
        colidx=np.tile(np.arange(128, dtype=np.float32), (128, 1)).copy(),
        ident=np.eye(128, dtype=np.float32),
        row0=ro_w[0], row1=ro_w[1], row2=ro_w[2], row3=ro_w[3],
    )
    if flags["ln_affine"]:
        shared["lnsc"] = np.repeat(ln_scale.reshape(L, 1, D), 128, axis=1).copy()
        shared["lnbs"] = np.repeat(ln_bias.reshape(L, 1, D), 128, axis=1).copy()
    if flags["wl_b"]:
        shared["wlb"] = np.repeat(wl_b.reshape(L, 1, D), 128, axis=1).copy()
    if flags["conv_b"]:
        shared["convb"] = np.repeat(conv_b.reshape(L, 1, D), 128, axis=1).copy()
    if flags["ro_b"]:
        for i, b in enumerate(ro_b):
            shared[f"rob{i}"] = np.repeat(b.reshape(1, -1), 128, axis=0).copy()

    for m in in_maps:
        m.update(shared)
    return in_maps, tiles_b, block_of_tile, T, T4, flags


# ----------------------------------------------------------------------------
# device program
# ----------------------------------------------------------------------------

def _build(tiles_b, block_of_tile, T, T4, flags):
    nc = bacc.Bacc("TRN2", target_bir_lowering=False, debug=False, num_devices=NC)

    # register LN epsilon as a const AP (activation float biases need one)
    _eps_t = nc.alloc_sbuf_tensor(f"const-float32-lneps", [128, 1], F32)
    nc.gpsimd.memset(_eps_t.ap(), LN_EPS)
    nc.const_aps.aps[(F32, LN_EPS)] = _eps_t.ap()
    nc.all_engine_barrier()

    # inputs
    TT = T4 * 4
    esrc = nc.dram_tensor("esrc", [128, TT], I32, kind="ExternalInput")
    emeta = nc.dram_tensor("emeta", [128, 2 * TT], F32, kind="ExternalInput")
    batchl = nc.dram_tensor("batchl", [128, NBLK], F32, kind="ExternalInput")
    xid = nc.dram_tensor("xid", [128, NBLK], I32, kind="ExternalInput")
    zrow = nc.dram_tensor("zrow", [128, L], I32, kind="ExternalInput")
    wlw = nc.dram_tensor("wlw", [L, 128, D], F32, kind="ExternalInput")
    convw = nc.dram_tensor("convw", [L, D, D], F32, kind="ExternalInput")
    nemb = nc.dram_tensor("nemb", [NTYPES, D], F32, kind="ExternalInput")
    colidx = nc.dram_tensor("colidx", [128, 128], F32, kind="ExternalInput")
    ident = nc.dram_tensor("ident", [128, 128], F32, kind="ExternalInput")
    row0 = nc.dram_tensor("row0", [6 * D, 768], F32, kind="ExternalInput")
    row1 = nc.dram_tensor("row1", [768, 384], F32, kind="ExternalInput")
    row2 = nc.dram_tensor("row2", [384, 192], F32, kind="ExternalInput")
    row3 = nc.dram_tensor("row3", [192, 1], F32, kind="ExternalInput")
    lnsc = lnbs = wlb = convb = None
    if flags["ln_affine"]:
        lnsc = nc.dram_tensor("lnsc", [L, 128, D], F32, kind="ExternalInput")
        lnbs = nc.dram_tensor("lnbs", [L, 128, D], F32, kind="ExternalInput")
    if flags["wl_b"]:
        wlb = nc.dram_tensor("wlb", [L, 128, D], F32, kind="ExternalInput")
    if flags["conv_b"]:
        convb = nc.dram_tensor("convb", [L, 128, D], F32, kind="ExternalInput")
    robs = None
    if flags["ro_b"]:
        robs = [
            nc.dram_tensor(f"rob{i}", [128, n], F32, kind="ExternalInput")
            for i, n in enumerate([768, 384, 192, 1])
        ]

    out = nc.dram_tensor("out", [B, 1], F32, kind="ExternalOutput")

    # LN groups of up to 4 blocks
    groups = []
    b0 = 0
    while b0 < NBLK:
        nb = min(4, NBLK - b0)
        groups.append((b0, nb))
        b0 += nb

    # tile ranges per block
    tstart = np.concatenate([[0], np.cumsum(tiles_b)]).astype(int)

    with tile.TileContext(nc) as tc:
        with (
            tc.tile_pool(name="dram", bufs=1, space="DRAM") as dram,
            tc.tile_pool(name="consts", bufs=1) as cpool,
            tc.tile_pool(name="lweights", bufs=2) as wpool,
        ):
            YDT = F16 if FP16_Y else F32
            hA = dram.tile([NSHARD, D], F32, tag="hA")
            hB = dram.tile([NSHARD, D], F32, tag="hB")
            y_c = dram.tile([NSHARD, D], YDT, tag="y_c")
            y_fulls = [
                dram.tile([NC * NSHARD, D], YDT, tag=f"y_full_{i}",
                          name=f"y_full_{i}", addr_space="Shared")
                for i in range(L)
            ]
            z_all = dram.tile([ZROWS, D], F32, tag="z_all")
            z_red = dram.tile([ZROWS, D], F32, tag="z_red", addr_space="Shared")

            colidx_t = cpool.tile([128, 128], F32, tag="colidx")
            nc.sync.dma_start(out=colidx_t[:], in_=colidx[:])
            ident_t = cpool.tile([128, 128], F32, tag="ident")
            nc.sync.dma_start(out=ident_t[:], in_=ident[:])
            # whole-kernel metadata, loaded once (partition-major)
            esrc_sb = cpool.tile([128, TT], I32, tag="esrc_sb")
            nc.sync.dma_start(out=esrc_sb[:], in_=esrc[:])
            emeta_sb = cpool.tile([128, 2 * TT], F32, tag="emeta_sb")
            nc.sync.dma_start(out=emeta_sb[:], in_=emeta[:])
            batchl_sb = cpool.tile([128, NBLK], F32, tag="batchl_sb")
            nc.sync.dma_start(out=batchl_sb[:], in_=batchl[:])
            xid_sb = cpool.tile([128, NBLK], I32, tag="xid_sb")
            nc.sync.dma_start(out=xid_sb[:], in_=xid[:])
            zrow_sb = cpool.tile([128, L], I32, tag="zrow_sb")
            nc.sync.dma_start(out=zrow_sb[:], in_=zrow[:])
            # pooling indicators, built once (gpsimd; reused across layers)
            ind_ts = []
            for b in range(NBLK):
                ind_b = cpool.tile([128, 128], F32, tag=f"ind{b}",
                                   name=f"ind{b}")
                nc.vector.tensor_tensor(
                    out=ind_b[:],
                    in0=batchl_sb[:, b:b + 1].to_broadcast([128, 128]),
                    in1=colidx_t[:],
                    op=ALU.is_equal,
                )
                ind_ts.append(ind_b)

            # ---------- zero z_all ----------
            with tc.tile_pool(name="zz", bufs=1) as zz:
                zt = zz.tile([128, D], F32)
                nc.vector.memset(zt[:], 0.0)
                for k in range(ZROWS // 128):
                    nc.sync.dma_start(
                        out=z_all[k * 128:(k + 1) * 128, :], in_=zt[:]
                    )

            # ---------- embedding ----------
            with tc.tile_pool(name="emb", bufs=4) as ep:
                for b in range(NBLK):
                    et = ep.tile([128, D], F32, tag="et")
                    nc.gpsimd.indirect_dma_start(
                        out=et[:], out_offset=None, in_=nemb[:],
                        in_offset=bass.IndirectOffsetOnAxis(
                            ap=xid_sb[:, b:b + 1], axis=0
                        ),
                    )
                    nc.sync.dma_start(
                        out=hA[b * 128:(b + 1) * 128, :], in_=et[:]
                    )

            # ---------- layers ----------
            with (
                tc.tile_pool(name="ln", bufs=2) as lp,
                tc.tile_pool(name="edge", bufs=4) as xp,
                tc.tile_pool(name="blk", bufs=3) as bp,
                tc.tile_pool(name="ps_nd", bufs=3, space="PSUM") as ps_nd,
                tc.tile_pool(name="ps_xt", bufs=2, space="PSUM") as ps_xt,
                tc.tile_pool(name="ps_h", bufs=1, space="PSUM") as ps_h,
                tc.tile_pool(name="ps_pool", bufs=1, space="PSUM") as ps_pool,
            ):
                for li in range(L):
                    h_in = hA if li % 2 == 0 else hB
                    h_out = hB if li % 2 == 0 else hA

                    wlw_t = wpool.tile([128, D], F32, tag="wlw")
                    nc.sync.dma_start(out=wlw_t[:], in_=wlw[li])
                    cw0 = wpool.tile([128, D], F32, tag="cw0")
                    nc.sync.dma_start(out=cw0[:], in_=convw[li, 0:128, :])
                    cw1 = wpool.tile([128, D], F32, tag="cw1")
                    nc.sync.dma_start(out=cw1[:], in_=convw[li, 128:256, :])
                    if flags["ln_affine"]:
                        lnsc_t = wpool.tile([128, D], F32, tag="lnsc")
                        nc.sync.dma_start(out=lnsc_t[:], in_=lnsc[li])
                        lnbs_t = wpool.tile([128, D], F32, tag="lnbs")
                        nc.sync.dma_start(out=lnbs_t[:], in_=lnbs[li])
                    if flags["wl_b"]:
                        wlb_t = wpool.tile([128, D], F32, tag="wlb")
                        nc.sync.dma_start(out=wlb_t[:], in_=wlb[li])
                    if flags["conv_b"]:
                        convb_t = wpool.tile([128, D], F32, tag="convb")
                        nc.sync.dma_start(out=convb_t[:], in_=convb[li])

                    # ---- LayerNorm: h_in -> y_c ----
                    for (gb, nb) in groups:
                        rows = slice(gb * 128, (gb + nb) * 128)
                        h_ap = h_in[rows, :].rearrange("(j p) d -> p j d", p=128)
                        ht = lp.tile([128, nb, D], F32, tag="ht")
                        nc.sync.dma_start(out=ht[:], in_=h_ap)
                        mu = lp.tile([128, 4], F32, tag="mu")
                        nc.vector.tensor_reduce(
                            out=mu[:, :nb], in_=ht[:], axis=mybir.AxisListType.X,
                            op=ALU.add,
                        )
                        mus = lp.tile([128, 4], F32, tag="mus")
                        nc.scalar.mul(mus[:, :nb], mu[:, :nb], 1.0 / D)
                        hc = lp.tile([128, nb, D], F32, tag="hc")
                        nc.vector.tensor_tensor(
                            out=hc[:], in0=ht[:],
                            in1=mus[:, :nb, None].to_broadcast([128, nb, D]),
                            op=ALU.subtract,
                        )
                        sq = lp.tile([128, nb, D], F32, tag="sq")
                        nc.scalar.square(sq[:], hc[:])
                        vs = lp.tile([128, 4], F32, tag="vs")
                        nc.vector.tensor_reduce(
                            out=vs[:, :nb], in_=sq[:], axis=mybir.AxisListType.X,
                            op=ALU.add,
                        )
                        sd = lp.tile([128, 4], F32, tag="sd")
                        nc.scalar.activation(
                            sd[:, :nb], vs[:, :nb], ACTF.Sqrt,
                            bias=LN_EPS, scale=1.0 / D,
                        )
                        rs = lp.tile([128, 4], F32, tag="rs")
                        nc.vector.reciprocal(rs[:, :nb], sd[:, :nb])
                        yt = lp.tile([128, nb, D], YDT, tag="yt")
                        nc.vector.tensor_tensor(
                            out=yt[:], in0=hc[:],
                            in1=rs[:, :nb, None].to_broadcast([128, nb, D]),
                            op=ALU.mult,
                        )
                        if flags["ln_affine"]:
                            nc.vector.tensor_tensor(
                                out=yt[:], in0=yt[:],
                                in1=lnsc_t[:, None, :].to_broadcast([128, nb, D]),
                                op=ALU.mult,
                            )
                            nc.vector.tensor_tensor(
                                out=yt[:], in0=yt[:],
                                in1=lnbs_t[:, None, :].to_broadcast([128, nb, D]),
                                op=ALU.add,
                            )
                        y_ap = y_c[rows, :].rearrange("(j p) d -> p j d", p=128)
                        nc.sync.dma_start(out=y_ap, in_=yt[:])

                    # ---- AllGather y ----
                    y_full = y_fulls[li]
                    nc.gpsimd.collective_compute(
                        "AllGather", ALU.bypass,
                        replica_groups=[list(range(NC))],
                        ins=[y_c[:].opt()], outs=[y_full[:].opt()],
                    )

                    # ---- edges + conv + pool ----
                    ppool = ps_pool.tile([128, D], F32, tag="ppool")
                    for b in range(NBLK):
                        nd = ps_nd.tile([128, 2 * D], F32, tag="nd")
                        t0, t1 = int(tstart[b]), int(tstart[b + 1])
                        for t in range(t0, t1):
                            ysrc = xp.tile([128, D], YDT, tag="ysrc")
                            nc.gpsimd.indirect_dma_start(
                                out=ysrc[:], out_offset=None, in_=y_full[:],
                                in_offset=bass.IndirectOffsetOnAxis(
                                    ap=esrc_sb[:, t:t + 1], axis=0
                                ),
                            )
                            st = xp.tile([128, 128], F32, tag="st")
                            nc.vector.tensor_tensor(
                                out=st[:],
                                in0=emeta_sb[:, t:t + 1].to_broadcast([128, 128]),
                                in1=colidx_t[:],
                                op=ALU.is_equal,
                            )
                            pre = xp.tile([128, D], F32, tag="pre")
                            nc.vector.scalar_tensor_tensor(
                                out=pre[:], in0=wlw_t[:],
                                scalar=emeta_sb[:, TT + t:TT + t + 1], in1=ysrc[:],
                                op0=ALU.mult, op1=ALU.add,
                            )
                            if flags["wl_b"]:
                                nc.vector.tensor_tensor(
                                    out=pre[:], in0=pre[:], in1=wlb_t[:],
                                    op=ALU.add,
                                )
                            msg = xp.tile([128, D], F32, tag="msg")
                            nc.scalar.activation(msg[:], pre[:], ACTF.Relu)
                            ev = xp.tile([128, 2 * D], F32, tag="ev")
                            nc.scalar.activation(ev[:, :D], msg[:], ACTF.Exp)
                            nc.vector.tensor_tensor(
                                out=ev[:, D:], in0=msg[:], in1=ev[:, :D],
                                op=ALU.mult,
                            )
                            nc.tensor.matmul(
                                out=nd[:], lhsT=st[:], rhs=ev[:],
                                start=(t == t0), stop=(t == t1 - 1),
                            )
                        # block post: softmax-agg + residual + conv + pool
                        td = bp.tile([128, D], F32, tag="td")
                        nc.vector.tensor_scalar_max(td[:], nd[:, :D], 1e-16)
                        yb = bp.tile([128, D], YDT, tag="yb")
                        nc.sync.dma_start(
                            out=yb[:], in_=y_c[b * 128:(b + 1) * 128, :]
                        )
                        rec = bp.tile([128, D], F32, tag="rec")
                        nc.vector.reciprocal_approx_fast(out=rec[:], in_=td[:])
                        xv = bp.tile([128, D], F32, tag="xv")
                        nc.vector.tensor_tensor(
                            out=xv[:], in0=nd[:, D:], in1=rec[:], op=ALU.mult
                        )
                        nc.vector.tensor_tensor(
                            out=xv[:], in0=xv[:], in1=yb[:], op=ALU.add
                        )
                        pxt = ps_xt.tile([128, D], F32, tag="pxt")
                        nc.tensor.transpose(
                            out=pxt[:, 0:128], in_=xv[:, 0:128], identity=ident_t[:]
                        )
                        nc.tensor.transpose(
                            out=pxt[:, 128:256], in_=xv[:, 128:256],
                            identity=ident_t[:],
                        )
                        xts = bp.tile([128, D], F32, tag="xts")
                        nc.vector.tensor_copy(out=xts[:], in_=pxt[:])
                        ph = ps_h.tile([128, D], F32, tag="ph")
                        nc.tensor.matmul(
                            out=ph[:], lhsT=xts[:, 0:128], rhs=cw0[:],
                            start=True, stop=False,
                        )
                        nc.tensor.matmul(
                            out=ph[:], lhsT=xts[:, 128:256], rhs=cw1[:],
                            start=False, stop=True,
                        )
                        if flags["conv_b"]:
                            nc.vector.tensor_tensor(
                                out=ph[:], in0=ph[:], in1=convb_t[:], op=ALU.add
                            )
                        hb = bp.tile([128, D], F32, tag="hb")
                        nc.scalar.activation(hb[:], ph[:], ACTF.Relu)
                        nc.sync.dma_start(
                            out=h_out[b * 128:(b + 1) * 128, :], in_=hb[:]
                        )
                        nc.tensor.matmul(
                            out=ppool[:], lhsT=ind_ts[b][:], rhs=hb[:],
                            start=(b == 0), stop=(b == NBLK - 1),
                        )
                    # pool -> z_all
                    zp = bp.tile([128, D], F32, tag="zp")
                    nc.vector.tensor_copy(out=zp[:], in_=ppool[:])
                    nc.gpsimd.indirect_dma_start(
                        out=z_all[:],
                        out_offset=bass.IndirectOffsetOnAxis(
                            ap=zrow_sb[:, li:li + 1], axis=0
                        ),
                        in_=zp[:], in_offset=None,
                    )

            # ---------- AllReduce z ----------
            nc.gpsimd.collective_compute(
                "AllReduce", ALU.add,
                replica_groups=[list(range(NC))],
                ins=[z_all[:].opt()], outs=[z_red[:].opt()],
            )

            # ---------- readout MLP (replicated) ----------
            with (
                tc.tile_pool(name="row", bufs=1) as rw,
                tc.tile_pool(name="ro", bufs=2) as ro,
                tc.tile_pool(name="ps_a", bufs=1, space="PSUM") as psa,
                tc.tile_pool(name="ps_b", bufs=1, space="PSUM") as psb,
                tc.tile_pool(name="ps_t", bufs=2, space="PSUM") as pst,
                tc.tile_pool(name="ps_o", bufs=1, space="PSUM") as pso,
            ):
                w0t = []
                for f in range(12):
                    w = rw.tile([128, 768], F32, tag=f"w0_{f}")
                    nc.sync.dma_start(out=w[:], in_=row0[f * 128:(f + 1) * 128, :])
                    w0t.append(w)
                w1t = []
                for f in range(6):
                    w = rw.tile([128, 384], F32, tag=f"w1_{f}")
                    nc.sync.dma_start(out=w[:], in_=row1[f * 128:(f + 1) * 128, :])
                    w1t.append(w)
                w2t = []
                for f in range(3):
                    w = rw.tile([128, 192], F32, tag=f"w2_{f}")
                    nc.sync.dma_start(out=w[:], in_=row2[f * 128:(f + 1) * 128, :])
                    w2t.append(w)
                w3a = rw.tile([128, 1], F32, tag="w3a")
                nc.sync.dma_start(out=w3a[:], in_=row3[0:128, :])
                w3b = rw.tile([64, 1], F32, tag="w3b")
                nc.sync.dma_start(out=w3b[:], in_=row3[128:192, :])
                robt = []
                if flags["ro_b"]:
                    for i, n in enumerate([768, 384, 192, 1]):
                        w = rw.tile([128, n], F32, tag=f"rob{i}")
                        nc.sync.dma_start(out=w[:], in_=robs[i][:])
                        robt.append(w)

                def transpose_chunk(src_ap, kdim):
                    """src_ap: [128, kdim] SBUF -> returns [kdim,128] SBUF tile."""
                    pt = pst.tile([128, 128], F32, tag="pt")
                    nc.tensor.transpose(
                        out=pt[:kdim, :], in_=src_ap, identity=ident_t[:]
                    )
                    ct = ro.tile([128, 128], F32, tag="ct")
                    nc.vector.tensor_copy(out=ct[:kdim, :], in_=pt[:kdim, :])
                    return ct

                for gb in range(4):
                    pA = psa.tile([128, 512], F32, tag="pA")
                    pB = psb.tile([128, 256], F32, tag="pB")
                    for f in range(12):
                        li, half = f // 2, f % 2
                        zc = ro.tile([128, 128], F32, tag="zc")
                        nc.sync.dma_start(
                            out=zc[:],
                            in_=z_red[
                                512 * li + 128 * gb: 512 * li + 128 * (gb + 1),
                                128 * half: 128 * (half + 1),
                            ],
                        )
                        zt = transpose_chunk(zc[:], 128)
                        nc.tensor.matmul(
                            out=pA[:], lhsT=zt[:], rhs=w0t[f][:, 0:512],
                            start=(f == 0), stop=(f == 11),
                        )
                        nc.tensor.matmul(
                            out=pB[:], lhsT=zt[:], rhs=w0t[f][:, 512:768],
                            start=(f == 0), stop=(f == 11),
                        )
                    z1 = ro.tile([128, 768], F32, tag="z1")
                    if flags["ro_b"]:
                        nc.vector.tensor_tensor(
                            out=pA[:], in0=pA[:], in1=robt[0][:, 0:512], op=ALU.add
                        )
                        nc.vector.tensor_tensor(
                            out=pB[:], in0=pB[:], in1=robt[0][:, 512:768], op=ALU.add
                        )
                    nc.scalar.activation(z1[:, 0:512], pA[:], ACTF.Gelu)
                    nc.scalar.activation(z1[:, 512:768], pB[:], ACTF.Gelu)

                    p2 = psa.tile([128, 384], F32, tag="p2")
                    for f in range(6):
                        zt = transpose_chunk(z1[:, 128 * f:128 * (f + 1)], 128)
                        nc.tensor.matmul(
                            out=p2[:], lhsT=zt[:], rhs=w1t[f][:],
                            start=(f == 0), stop=(f == 5),
                        )
                    if flags["ro_b"]:
                        nc.vector.tensor_tensor(
                            out=p2[:], in0=p2[:], in1=robt[1][:], op=ALU.add
                        )
                    z2 = ro.tile([128, 384], F32, tag="z2")
                    nc.scalar.activation(z2[:], p2[:], ACTF.Gelu)

                    p3 = psb.tile([128, 192], F32, tag="p3")
                    for f in range(3):
                        zt = transpose_chunk(z2[:, 128 * f:128 * (f + 1)], 128)
                        nc.tensor.matmul(
                            out=p3[:], lhsT=zt[:], rhs=w2t[f][:],
                            start=(f == 0), stop=(f == 2),
                        )
                    if flags["ro_b"]:
                        nc.vector.tensor_tensor(
                            out=p3[:], in0=p3[:], in1=robt[2][:], op=ALU.add
                        )
                    z3 = ro.tile([128, 192], F32, tag="z3")
                    nc.scalar.activation(z3[:], p3[:], ACTF.Gelu)

                    po = pso.tile([128, 1], F32, tag="po")
                    zt = transpose_chunk(z3[:, 0:128], 128)
                    nc.tensor.matmul(
                        out=po[:], lhsT=zt[:], rhs=w3a[:],
                        start=True, stop=False,
                    )
                    zt = transpose_chunk(z3[:, 128:192], 64)
                    nc.tensor.matmul(
                        out=po[:], lhsT=zt[:64, :], rhs=w3b[:],
                        start=False, stop=True,
                    )
                    oc = ro.tile([128, 1], F32, tag="oc")
                    if flags["ro_b"]:
                        nc.vector.tensor_tensor(
                            out=po[:], in0=po[:], in1=robt[3][:], op=ALU.add
                        )
                    nc.vector.tensor_copy(out=oc[:], in_=po[:])
                    nc.sync.dma_start(
                        out=out[128 * gb:128 * (gb + 1), :], in_=oc[:]
                    )

    nc.compile()
    return nc


# ----------------------------------------------------------------------------
# entry point
# ----------------------------------------------------------------------------

def kernel(**inputs):
    in_maps, tiles_b, block_of_tile, T, T4, flags = _prep(inputs)
    key = (tuple(tiles_b.tolist()), tuple(sorted(flags.items())))
    if key not in _prog_cache:
        _prog_cache[key] = _build(tiles_b, block_of_tile, T, T4, flags)
    nc = _prog_cache[key]

    kwargs = {}
    if TRACE:
        kwargs = dict(trace=True, trace_cores=TRACE_CORES)
    res = run_bass_kernel_spmd(nc, in_maps, list(range(NC)), **kwargs)
    LAST_RESULT["exec_time_ns"] = getattr(res, "exec_time_ns", None)
    LAST_RESULT["res"] = res
    return np.asarray(res.results[0]["out"], dtype=np.float32)
